# revision 33
# baseline (speedup 1.0000x reference)
"""Trainium2 Bass kernel for nn_DistillingLayer: per-channel shared-weight
Conv1d(k=3, stride=2, pad=1) + ELU + MaxPool1d(k=3, stride=2, pad=1) over
x:(16, 4096, 512) f32 -> out:(16, 1024, 512) f32.

Strategy (TensorE conv + fp16 streaming, DMA-roofline focused)
--------------------------------------------------------------
- Data-parallel over batch: 8 cores x 2 batches each. No communication.
- The kernel is HBM-bound: it must read 16.8 MB of f32 input per core
  (~330-420 GB/s/NC observed). Everything else is arranged so the SWDGE
  input stream never waits: measured e2e ~= preamble (9 us) + stream
  (42-52 us) + drain tail (~12 us) + exit barrier (~9 us).
- Layout: L in the SBUF free dimension; one tile per batch; partition p
  owns exactly rows [32p, 32p+32) x D=512 channels (no halo reload).
- Input DMAs run on the gpsimd (SWDGE) queue and cast f32->fp16 in the
  SDMA datapath. The two tiles' row-chunks are interleaved and ordered
  so compute dependencies unlock smoothly; the final chunk is a single
  row feeding one short conv+pool+store chain.
- Conv runs on the otherwise-idle TensorE: diag(w_k) stationaries make
  matmuls partition-preserving elementwise scales, accumulated in PSUM
  (fp32); w_k*eye(k=1) shift stationaries route partition p-1's last
  rows into p's two boundary conv rows, replacing a 3-row halo reload
  (drops input traffic 8.6%). ScalarE evicts PSUM -> fp16 SBUF with
  the conv bias (+1, see below) folded into the activation bias.
- ELU is monotonic, so maxpool commutes: pool the pre-activation conv
  rows (two VectorE 2x tensor_tensor max passes), then ELU once on the
  pooled rows. The whole pipeline computes out+1 (conv bias shifted by
  +1; host subtracts 1): out+1 = max(exp(min(v,0)), v+1) needs only a
  4x dual-op tensor_scalar (min(P-1,0)), one ScalarE Exp, and a 2x
  tensor_tensor max. scalar_tensor_tensor is avoided everywhere (it
  only has a 1x DVE uop).
- Outputs are stored as fp16 via the sync (HWDGE) queue - it runs in
  parallel with the SWDGE input queue, so stores never block loads -
  and upcast to f32 on the host (absmax-scaled error ~8e-4, gate 2e-2).
- Weights/bias are baked as immediates; the compiled module is cached
  per (w, b) value.

Toolchain workaround (see inline comment): a BIR post-pass splits
multi-wait instructions — this walrus build allows one sync wait per
instruction.
"""

import json as _json
import os
import sys

import numpy as np

for _p in ("/opt/trn_rl_repo", "/root/.axon_site/_ro/trn_rl_repo"):
    if os.path.isdir(_p) and _p not in sys.path:
        sys.path.append(_p)

import concourse.bass as bass
import concourse.bass2jax as bass2jax
import concourse.bass_utils as bass_utils
import concourse.mybir as mybir
from concourse.bass_utils import run_bass_kernel_spmd
from concourse.tile import TileContext

# ---------------------------------------------------------------------------
# REQUIRED workaround: this container's walrus build rejects instructions
# carrying more than one sync wait ("Too many sync wait commands" in
# setupSyncWait). Tile's scheduler freely attaches several waits to one
# instruction, so post-process the BIR JSON before compile: hoist all but the
# last wait onto same-engine NoOps inserted just before the instruction
# (per-engine program order makes sequential waits equivalent to a
# multi-wait).
# ---------------------------------------------------------------------------

_orig_compile_bir_kernel = bass_utils.compile_bir_kernel


def _split_multi_waits(bir_json: bytes) -> bytes:
    j = _json.loads(bir_json)
    ctr = 0
    changed = False
    for fn in j["functions"]:
        for bb in fn["blocks"]:
            out = []
            for ins in bb["instructions"]:
                si = ins.get("sync_info")
                waits = (si.get("on_wait") or []) if si else []
                if len(waits) > 1:
                    changed = True
                    for w in waits[:-1]:
                        ctr += 1
                        out.append(
                            {
                                "debug": ins.get("debug", 0),
                                "engine": ins["engine"],
                                "ins": [],
                                "outs": [],
                                "name": f"waitsplit-{ctr}",
                                "opcode": "NoOp",
                                "text_hint": "waitsplit",
                                "sync_info": {"on_update": [], "on_wait": [w]},
                            }
                        )
                    si["on_wait"] = [waits[-1]]
                out.append(ins)
            bb["instructions"] = out
    if not changed:
        return bir_json
    return _json.dumps(j).encode()


def _patched_compile_bir_kernel(bir_json, tmpdir, neff_name="file.neff"):
    return _orig_compile_bir_kernel(_split_multi_waits(bir_json), tmpdir, neff_name)


bass_utils.compile_bir_kernel = _patched_compile_bir_kernel
bass2jax.compile_bir_kernel = _patched_compile_bir_kernel

# The first TileContext exit barrier's per-engine drains are redundant (the
# tail waits already cover all completions); use the cheap sequencer-level
# variant there. The SECOND barrier stays full — its drains restore
# engine/queue state so the loaded NEFF can re-execute.
try:
    from concourse.vector_clock import ScopedClock as _ScopedClock

    def _tail_drain_and_barrier(self, tick_clock, wait_clock):
        drain_inst = self.nc.sync.drain()
        wait_clock.add_sem_waits(
            drain_inst.ins, _ScopedClock({None: tick_clock.global_clock})
        )
        self.nc.all_engine_barrier(sem_only=True)
        assert self.sems is not None
        popped = self.nc._tile_sem_poison_stack.pop()
        assert popped is self._sem_poison
        # Skip the device-side dma_reset/sem_clear of
        # clear_and_free_semaphores: the bass preamble re-clears the full
        # semaphore range at the start of every execution, so exit-time
        # clears are redundant (re-execution correctness verified by
        # running the kernel twice in one process). Keep the host-side
        # allocator bookkeeping.
        sem_nums = [s.num for s in self.sems.allocated().values()]
        self.nc._state.prepend_free_semaphores(sem_nums)
        for poison_set in self.nc._tile_sem_poison_stack:
            poison_set.update(sem_nums)
        self.nc.all_engine_barrier(sem_only=True)

    TileContext._drain_and_barrier = _tail_drain_and_barrier
except Exception:
    pass

# ---------------------------------------------------------------------------

N_CORES = 8
B, L, D = 16, 4096, 512
BPC = B // N_CORES  # batches per core
LC = L // 2         # conv output length
LP = LC // 2        # pool output length
S = 32              # input L-rows owned per partition (128 * 32 = 4096)
Q = S // 2 + 1      # conv rows per partition (incl. 1 left-halo conv row)
JT = S // 4         # pool-output rows per partition

F32 = mybir.dt.float32
F16 = mybir.dt.float16
ALU = mybir.AluOpType
AF = mybir.ActivationFunctionType

_cache: dict = {}

# Exposed for test harnesses: the BassKernelResults of the last run.
LAST_RESULT = None


def _build(w0: float, w1: float, w2: float, bias: float) -> bass.Bass:
    nc = bass.Bass()
    # x is the raw unpadded input: partition p of a tile owns exactly rows
    # [32p, 32p+32) -- no halo rows are loaded (see the shift matrices).
    x = nc.dram_tensor("x", [BPC, L, D], F32, kind="ExternalInput")
    # wd holds six 128x128 stationary matrices (fp16): w_k * I for k=0..2
    # (diag(w) @ X == w * X elementwise, partition-preserving) and
    # w_k * eye(k=1), whose matmul routes partition p-1's row to partition
    # p -- used for the two boundary conv rows whose taps live in the
    # previous partition, replacing a 3-row halo reload from HBM.
    wd = nc.dram_tensor("wd", [128, 6 * 128], F16, kind="ExternalInput")
    y = nc.dram_tensor("y", [BPC, LP, D], F16, kind="ExternalOutput")

    xrow = D              # elements per L-row
    xbat = L * D          # elements per input batch
    ybat = LP * D

    with TileContext(nc) as tc:
        with (
            tc.tile_pool(name="xp", bufs=2) as xp,
            tc.tile_pool(name="yp", bufs=2) as yp,
            tc.tile_pool(name="wp", bufs=1) as wp,
            tc.tile_pool(name="cp", bufs=2, space="PSUM") as cp,
            tc.tile_pool(name="pp", bufs=2) as pp,
            tc.tile_pool(name="rp", bufs=2) as rp,
        ):
            # The six stationary matrices, loaded once up front on the sync
            # (HWDGE) queue so the SWDGE input stream is not delayed.
            WD = wp.tile([128, 6 * 128], F16)
            nc.sync.dma_start(
                out=WD[:, :],
                in_=bass.AP(wd, 0, [[6 * 128, 128], [1, 6 * 128]]),
            )
            # Input row-chunks, conv q-waves and pool j-segments are aligned
            # so each conv wave only needs already-landed chunks (conv row q
            # taps local rows [2qa, 2qb+1)) and each pool segment only needs
            # finished conv rows (q in [2ja, 2jb+1)). The two batch tiles'
            # chunks are INTERLEAVED in the SWDGE stream, so each tile's
            # compute spreads across the whole stream and only one short
            # wave+pool chain hangs off the final chunk (tile 1's pool seg
            # (7,8) via conv q=16).
            # Conv wave (qa,qb) with qa>=2 taps local rows [2qa-3, 2qb-2);
            # the boundary wave (0,2) taps the previous partition's rows
            # 29-31 (routed cross-partition by the shift stationaries) plus
            # local rows 0-1. Rows 29-31 are loaded FIRST, so both the
            # boundary wave and q=16 unlock early; the final chunk is the
            # single row 28, which only conv row q=15 needs -- the
            # post-stream tail is one 3-matmul + FD-512 evict/pool/store
            # chain per tile.
            chunks = [(29, 32), (0, 11), (11, 19), (19, 25), (25, 27), (27, 28), (28, 29)]

            tiles = []
            for b in range(BPC):
                # Partition p holds exactly its own rows [32p, 32p+32).
                X = xp.tile([128, S * D], F16)
                Y = yp.tile([128, Q * D], F16)
                P = pp.tile([128, JT * D], F16)
                R = rp.tile([128, JT * D], F16)
                tiles.append((b, X, Y, P, R))

            for ci in range(len(chunks)):
                r0, r1 = chunks[ci]
                for b, X, Y, P, R in tiles:
                    nc.gpsimd.dma_start(
                        out=X[:, r0 * D : r1 * D],
                        in_=bass.AP(
                            x,
                            b * xbat + r0 * xrow,
                            [[S * xrow, 128], [1, (r1 - r0) * xrow]],
                        ),
                    )

            # conv, shifted by +1 (the host subtracts 1 from the final
            # output): partition p's conv row q (local) is
            # c[16p - 1 + q] = w0*x[2q-3] + w1*x[2q-2] + w2*x[2q-1] + bias+1
            # (x indices local to the partition's 32-row strip).
            # The taps run on the otherwise-idle TensorE: diag(w_k) as the
            # stationary makes a matmul a partition-preserving elementwise
            # scale, and the three taps accumulate in a PSUM bank (fp32).
            # Matmuls are grouped by tap so the stationary is swapped 3x per
            # wave, not per row. ScalarE then evicts PSUM -> fp16 SBUF,
            # folding in bias+1 via the activation bias.
            def emit_wave(tile, qa, qb):
                b, X, Y, P, R = tile
                nq = qb - qa
                Xv = X[:, :].rearrange("p (r d) -> p r d", d=D)
                C4 = cp.tile([128, nq * 512], F32, tag="cw")
                for k in range(3):
                    Wk = WD[:, k * 128 : (k + 1) * 128]
                    for q in range(qa, qb):
                        nc.tensor.matmul(
                            C4[:, (q - qa) * 512 : (q - qa + 1) * 512],
                            Wk,
                            Xv[:, 2 * q - 3 + k, :],
                            start=(k == 0),
                            stop=(k == 2),
                        )
                nc.scalar.activation(
                    Y[:, qa * D : qb * D], C4[:, :], AF.Copy, bias=bias + 1.0
                )

            # Boundary wave (0,2): conv rows q=0,1 of partition p tap rows
            # 29-31 of partition p-1 (routed by the shift stationaries
            # WD[:, (3+k)*128:]) plus local rows 0,1. Partition 0's q=0 gets
            # all-zero shift input (= the pool's excluded left pad), so it
            # is overwritten with -inf after eviction; its q=1 correctly
            # sees zero for the conv's left pad x[-1].
            def emit_boundary(tile):
                b, X, Y, P, R = tile
                Xv = X[:, :].rearrange("p (r d) -> p r d", d=D)
                C2 = cp.tile([128, 2 * 512], F32, tag="cw")
                for k in range(3):
                    Sk = WD[:, (3 + k) * 128 : (4 + k) * 128]
                    nc.tensor.matmul(
                        C2[:, 0:512],
                        Sk,
                        Xv[:, 29 + k, :],
                        start=(k == 0),
                        stop=(k == 2),
                    )
                nc.tensor.matmul(
                    C2[:, 512:1024], WD[:, 3 * 128 : 4 * 128], Xv[:, 31, :],
                    start=True, stop=False,
                )
                nc.tensor.matmul(
                    C2[:, 512:1024], WD[:, 1 * 128 : 2 * 128], Xv[:, 0, :],
                    start=False, stop=False,
                )
                nc.tensor.matmul(
                    C2[:, 512:1024], WD[:, 2 * 128 : 3 * 128], Xv[:, 1, :],
                    start=False, stop=True,
                )
                nc.scalar.activation(
                    Y[:, 0 : 2 * D], C2[:, :], AF.Copy, bias=bias + 1.0
                )
                nc.vector.memset(Y[0:1, 0:D], float("-inf"))

            # maxpool (pre-activation; ELU is monotonic), all shifted +1:
            # P[8p + j] = max(y3[2j], y3[2j+1], y3[2j+2]) = v + 1; then
            # out+1 = max(exp(min(v,0)), v+1), via one 4x dual-op
            # tensor_scalar (m = min(P-1, 0)), one ScalarE Exp, one 2x
            # tensor_tensor max. Stores go out fp16 on the sync (HWDGE)
            # queue, parallel to the SWDGE input queue.
            def emit_pool(tile, ja, jb):
                b, X, Y, P, R = tile
                y3 = Y[:, :].rearrange("p (q d) -> p q d", d=D)
                p3 = P[:, :].rearrange("p (j d) -> p j d", d=D)
                ps = p3[:, ja:jb, :]
                pf = P[:, ja * D : jb * D]
                rs = R[:, ja * D : jb * D]
                nc.vector.tensor_tensor(
                    ps,
                    y3[:, 2 * ja : 2 * jb - 1 : 2, :],
                    y3[:, 2 * ja + 1 : 2 * jb : 2, :],
                    op=ALU.max,
                )
                nc.vector.tensor_tensor(
                    ps, ps, y3[:, 2 * ja + 2 : 2 * jb + 1 : 2, :], op=ALU.max
                )
                nc.vector.tensor_scalar(
                    rs, pf, -1.0, 0.0, op0=ALU.add, op1=ALU.min
                )
                nc.scalar.activation(rs, rs, AF.Exp)
                nc.vector.tensor_tensor(rs, rs, pf, op=ALU.max)
                nc.sync.dma_start(
                    out=bass.AP(
                        y,
                        b * ybat + ja * xrow,
                        [[JT * xrow, 128], [1, (jb - ja) * xrow]],
                    ),
                    in_=rs,
                )

            # Emission follows chunk-readiness order: the sequenced engines
            # (ScalarE evicts in particular) execute in program order, so an
            # early-ready op emitted after a late-gated one head-of-line
            # blocks the engine.
            stages = [
                ("w", 16, 17),   # ready after chunk (29,32)
                ("w", 2, 6),     # ready after chunk (0,11)
                ("bd", 0, 0),    # ready after chunks (29,32)+(0,11)
                ("p", 0, 1),
                ("w", 6, 10),    # after chunk (11,19)
                ("p", 1, 4),
                ("w", 10, 13),   # after chunk (19,25)
                ("p", 4, 6),
                ("w", 13, 14),   # after chunk (25,27)
                ("w", 14, 15),   # after chunk (27,28)
                ("p", 6, 7),
                ("w", 15, 16),   # after the final chunk (28,29)
                ("p", 7, 8),
            ]
            for kind, a_, b_ in stages:
                for tile in tiles:
                    if kind == "w":
                        emit_wave(tile, a_, b_)
                    elif kind == "bd":
                        emit_boundary(tile)
                    else:
                        emit_pool(tile, a_, b_)
    return nc


def kernel(x: np.ndarray, w: np.ndarray, b: np.ndarray) -> np.ndarray:
    global LAST_RESULT
    w = np.asarray(w, dtype=np.float32)
    bb = np.asarray(b, dtype=np.float32)
    key = (float(w[0]), float(w[1]), float(w[2]), float(bb[0]))
    if key not in _cache:
        _cache[key] = _build(*key)
    nc = _cache[key]

    x = np.asarray(x, dtype=np.float32)
    assert x.shape == (B, L, D), x.shape
    wdiag = np.concatenate(
        [np.eye(128, dtype=np.float16) * np.float16(w[k]) for k in range(3)]
        + [
            np.eye(128, k=1, dtype=np.float16) * np.float16(w[k])
            for k in range(3)
        ],
        axis=1,
    )
    in_maps = [
        {
            "x": np.ascontiguousarray(x[c * BPC : (c + 1) * BPC]),
            "wd": wdiag,
        }
        for c in range(N_CORES)
    ]
    res = run_bass_kernel_spmd(nc, in_maps, core_ids=list(range(N_CORES)))
    LAST_RESULT = res
    out = np.concatenate([r["y"] for r in res.results], axis=0)
    # device computes out+1 in fp16 (see _build); undo the shift here
    return out.astype(np.float32) - 1.0


# revision 35
# speedup vs baseline: 1.0009x; 1.0009x over previous
"""Trainium2 Bass kernel for nn_DistillingLayer: per-channel shared-weight
Conv1d(k=3, stride=2, pad=1) + ELU + MaxPool1d(k=3, stride=2, pad=1) over
x:(16, 4096, 512) f32 -> out:(16, 1024, 512) f32.

Strategy (TensorE conv + fp16 streaming, DMA-roofline focused)
--------------------------------------------------------------
- Data-parallel over batch: 8 cores x 2 batches each. No communication.
- The kernel is HBM-bound: it must read 16.8 MB of f32 input per core
  (~330-420 GB/s/NC observed). Everything else is arranged so the SWDGE
  input stream never waits: measured e2e ~= preamble (9 us) + stream
  (42-52 us) + drain tail (~12 us) + exit barrier (~9 us).
- Layout: L in the SBUF free dimension; one tile per batch; partition p
  owns exactly rows [32p, 32p+32) x D=512 channels (no halo reload).
- Input DMAs run on the gpsimd (SWDGE) queue and cast f32->fp16 in the
  SDMA datapath. The two tiles' row-chunks are interleaved and ordered
  so compute dependencies unlock smoothly; the final chunk is a single
  row feeding one short conv+pool+store chain.
- Conv runs on the otherwise-idle TensorE: diag(w_k) stationaries make
  matmuls partition-preserving elementwise scales, accumulated in PSUM
  (fp32); w_k*eye(k=1) shift stationaries route partition p-1's last
  rows into p's two boundary conv rows, replacing a 3-row halo reload
  (drops input traffic 8.6%). ScalarE evicts PSUM -> fp16 SBUF with
  the conv bias (+1, see below) folded into the activation bias.
- ELU is monotonic, so maxpool commutes: pool the pre-activation conv
  rows (two VectorE 2x tensor_tensor max passes), then ELU once on the
  pooled rows. The whole pipeline computes out+1 (conv bias shifted by
  +1; host subtracts 1): out+1 = max(exp(min(v,0)), v+1) needs only a
  4x dual-op tensor_scalar (min(P-1,0)), one ScalarE Exp, and a 2x
  tensor_tensor max. scalar_tensor_tensor is avoided everywhere (it
  only has a 1x DVE uop).
- Outputs are stored as fp16 via the sync (HWDGE) queue - it runs in
  parallel with the SWDGE input queue, so stores never block loads -
  and upcast to f32 on the host (absmax-scaled error ~8e-4, gate 2e-2).
- Weights/bias are baked as immediates; the compiled module is cached
  per (w, b) value.

Toolchain workaround (see inline comment): a BIR post-pass splits
multi-wait instructions — this walrus build allows one sync wait per
instruction.
"""

import json as _json
import os
import sys

import numpy as np

for _p in ("/opt/trn_rl_repo", "/root/.axon_site/_ro/trn_rl_repo"):
    if os.path.isdir(_p) and _p not in sys.path:
        sys.path.append(_p)

import concourse.bass as bass
import concourse.bass2jax as bass2jax
import concourse.bass_utils as bass_utils
import concourse.mybir as mybir
from concourse.bass_utils import run_bass_kernel_spmd
from concourse.tile import TileContext

# ---------------------------------------------------------------------------
# REQUIRED workaround: this container's walrus build rejects instructions
# carrying more than one sync wait ("Too many sync wait commands" in
# setupSyncWait). Tile's scheduler freely attaches several waits to one
# instruction, so post-process the BIR JSON before compile: hoist all but the
# last wait onto same-engine NoOps inserted just before the instruction
# (per-engine program order makes sequential waits equivalent to a
# multi-wait).
# ---------------------------------------------------------------------------

_orig_compile_bir_kernel = bass_utils.compile_bir_kernel


def _split_multi_waits(bir_json: bytes) -> bytes:
    j = _json.loads(bir_json)
    ctr = 0
    changed = False
    for fn in j["functions"]:
        for bb in fn["blocks"]:
            out = []
            for ins in bb["instructions"]:
                si = ins.get("sync_info")
                waits = (si.get("on_wait") or []) if si else []
                if len(waits) > 1:
                    changed = True
                    for w in waits[:-1]:
                        ctr += 1
                        out.append(
                            {
                                "debug": ins.get("debug", 0),
                                "engine": ins["engine"],
                                "ins": [],
                                "outs": [],
                                "name": f"waitsplit-{ctr}",
                                "opcode": "NoOp",
                                "text_hint": "waitsplit",
                                "sync_info": {"on_update": [], "on_wait": [w]},
                            }
                        )
                    si["on_wait"] = [waits[-1]]
                out.append(ins)
            bb["instructions"] = out
    if not changed:
        return bir_json
    return _json.dumps(j).encode()


def _patched_compile_bir_kernel(bir_json, tmpdir, neff_name="file.neff"):
    return _orig_compile_bir_kernel(_split_multi_waits(bir_json), tmpdir, neff_name)


bass_utils.compile_bir_kernel = _patched_compile_bir_kernel
bass2jax.compile_bir_kernel = _patched_compile_bir_kernel

# The first TileContext exit barrier's per-engine drains are redundant (the
# tail waits already cover all completions); use the cheap sequencer-level
# variant there. The SECOND barrier stays full — its drains restore
# engine/queue state so the loaded NEFF can re-execute.
try:
    from concourse.vector_clock import ScopedClock as _ScopedClock

    def _tail_drain_and_barrier(self, tick_clock, wait_clock):
        drain_inst = self.nc.sync.drain()
        wait_clock.add_sem_waits(
            drain_inst.ins, _ScopedClock({None: tick_clock.global_clock})
        )
        self.nc.all_engine_barrier(sem_only=True)
        assert self.sems is not None
        popped = self.nc._tile_sem_poison_stack.pop()
        assert popped is self._sem_poison
        # Skip the device-side dma_reset/sem_clear of
        # clear_and_free_semaphores: the bass preamble re-clears the full
        # semaphore range at the start of every execution, so exit-time
        # clears are redundant (re-execution correctness verified by
        # running the kernel twice in one process). Keep the host-side
        # allocator bookkeeping.
        sem_nums = [s.num for s in self.sems.allocated().values()]
        self.nc._state.prepend_free_semaphores(sem_nums)
        for poison_set in self.nc._tile_sem_poison_stack:
            poison_set.update(sem_nums)
        self.nc.all_engine_barrier(sem_only=True)

    TileContext._drain_and_barrier = _tail_drain_and_barrier
except Exception:
    pass

# ---------------------------------------------------------------------------

N_CORES = 8
B, L, D = 16, 4096, 512
BPC = B // N_CORES  # batches per core
LC = L // 2         # conv output length
LP = LC // 2        # pool output length
S = 32              # input L-rows owned per partition (128 * 32 = 4096)
Q = S // 2 + 1      # conv rows per partition (incl. 1 left-halo conv row)
JT = S // 4         # pool-output rows per partition

F32 = mybir.dt.float32
F16 = mybir.dt.float16
ALU = mybir.AluOpType
AF = mybir.ActivationFunctionType

_cache: dict = {}

# Exposed for test harnesses: the BassKernelResults of the last run.
LAST_RESULT = None


def _build(w0: float, w1: float, w2: float, bias: float) -> bass.Bass:
    nc = bass.Bass()
    # x is the raw unpadded input: partition p of a tile owns exactly rows
    # [32p, 32p+32) -- no halo rows are loaded (see the shift matrices).
    x = nc.dram_tensor("x", [BPC, L, D], F32, kind="ExternalInput")
    # wd holds six 128x128 stationary matrices (fp16): w_k * I for k=0..2
    # (diag(w) @ X == w * X elementwise, partition-preserving) and
    # w_k * eye(k=1), whose matmul routes partition p-1's row to partition
    # p -- used for the two boundary conv rows whose taps live in the
    # previous partition, replacing a 3-row halo reload from HBM.
    wd = nc.dram_tensor("wd", [128, 6 * 128], F16, kind="ExternalInput")
    y = nc.dram_tensor("y", [BPC, LP, D], F16, kind="ExternalOutput")

    xrow = D              # elements per L-row
    xbat = L * D          # elements per input batch
    ybat = LP * D

    with TileContext(nc) as tc:
        with (
            tc.tile_pool(name="xp", bufs=2) as xp,
            tc.tile_pool(name="yp", bufs=2) as yp,
            tc.tile_pool(name="wp", bufs=1) as wp,
            tc.tile_pool(name="cp", bufs=2, space="PSUM") as cp,
            tc.tile_pool(name="pp", bufs=2) as pp,
            tc.tile_pool(name="rp", bufs=2) as rp,
        ):
            # The six stationary matrices, loaded once up front on the sync
            # (HWDGE) queue so the SWDGE input stream is not delayed.
            WD = wp.tile([128, 6 * 128], F16)
            nc.sync.dma_start(
                out=WD[:, :],
                in_=bass.AP(wd, 0, [[6 * 128, 128], [1, 6 * 128]]),
            )
            # Input row-chunks, conv q-waves and pool j-segments are aligned
            # so each conv wave only needs already-landed chunks (conv row q
            # taps local rows [2qa, 2qb+1)) and each pool segment only needs
            # finished conv rows (q in [2ja, 2jb+1)). The two batch tiles'
            # chunks are INTERLEAVED in the SWDGE stream, so each tile's
            # compute spreads across the whole stream and only one short
            # wave+pool chain hangs off the final chunk (tile 1's pool seg
            # (7,8) via conv q=16).
            # Conv wave (qa,qb) with qa>=2 taps local rows [2qa-3, 2qb-2);
            # the boundary wave (0,2) taps the previous partition's rows
            # 29-31 (routed cross-partition by the shift stationaries) plus
            # local rows 0-1. Rows 29-31 are loaded FIRST, so both the
            # boundary wave and q=16 unlock early; the final chunk is the
            # single row 28, which only conv row q=15 needs -- the
            # post-stream tail is one 3-matmul + FD-512 evict/pool/store
            # chain per tile.
            chunks = [(29, 32), (0, 11), (11, 19), (19, 25), (25, 27), (27, 28), (28, 29)]

            tiles = []
            for b in range(BPC):
                # Partition p holds exactly its own rows [32p, 32p+32).
                X = xp.tile([128, S * D], F16)
                Y = yp.tile([128, Q * D], F16)
                P = pp.tile([128, JT * D], F16)
                R = rp.tile([128, JT * D], F16)
                tiles.append((b, X, Y, P, R))

            for ci in range(len(chunks)):
                r0, r1 = chunks[ci]
                for b, X, Y, P, R in tiles:
                    nc.gpsimd.dma_start(
                        out=X[:, r0 * D : r1 * D],
                        in_=bass.AP(
                            x,
                            b * xbat + r0 * xrow,
                            [[S * xrow, 128], [1, (r1 - r0) * xrow]],
                        ),
                    )

            # conv, shifted by +1 (the host subtracts 1 from the final
            # output): partition p's conv row q (local) is
            # c[16p - 1 + q] = w0*x[2q-3] + w1*x[2q-2] + w2*x[2q-1] + bias+1
            # (x indices local to the partition's 32-row strip).
            # The taps run on the otherwise-idle TensorE: diag(w_k) as the
            # stationary makes a matmul a partition-preserving elementwise
            # scale, and the three taps accumulate in a PSUM bank (fp32).
            # Matmuls are grouped by tap so the stationary is swapped 3x per
            # wave, not per row. ScalarE then evicts PSUM -> fp16 SBUF,
            # folding in bias+1 via the activation bias.
            def emit_wave(tile, qa, qb, evict_on_v=False):
                b, X, Y, P, R = tile
                nq = qb - qa
                Xv = X[:, :].rearrange("p (r d) -> p r d", d=D)
                C4 = cp.tile([128, nq * 512], F32, tag="cw")
                for k in range(3):
                    Wk = WD[:, k * 128 : (k + 1) * 128]
                    for q in range(qa, qb):
                        nc.tensor.matmul(
                            C4[:, (q - qa) * 512 : (q - qa + 1) * 512],
                            Wk,
                            Xv[:, 2 * q - 3 + k, :],
                            start=(k == 0),
                            stop=(k == 2),
                        )
                if evict_on_v:
                    # Late waves evict on DVE (bias via tensor_scalar add)
                    # so the stream-end eviction+exp backlog splits across
                    # ScalarE and VectorE instead of serializing on ScalarE.
                    nc.vector.tensor_scalar(
                        Y[:, qa * D : qb * D], C4[:, :], bias + 1.0, None,
                        op0=ALU.add,
                    )
                else:
                    nc.scalar.activation(
                        Y[:, qa * D : qb * D], C4[:, :], AF.Copy, bias=bias + 1.0
                    )

            # Boundary wave (0,2): conv rows q=0,1 of partition p tap rows
            # 29-31 of partition p-1 (routed by the shift stationaries
            # WD[:, (3+k)*128:]) plus local rows 0,1. Partition 0's q=0 gets
            # all-zero shift input (= the pool's excluded left pad), so it
            # is overwritten with -inf after eviction; its q=1 correctly
            # sees zero for the conv's left pad x[-1].
            def emit_boundary(tile):
                b, X, Y, P, R = tile
                Xv = X[:, :].rearrange("p (r d) -> p r d", d=D)
                C2 = cp.tile([128, 2 * 512], F32, tag="cw")
                for k in range(3):
                    Sk = WD[:, (3 + k) * 128 : (4 + k) * 128]
                    nc.tensor.matmul(
                        C2[:, 0:512],
                        Sk,
                        Xv[:, 29 + k, :],
                        start=(k == 0),
                        stop=(k == 2),
                    )
                nc.tensor.matmul(
                    C2[:, 512:1024], WD[:, 3 * 128 : 4 * 128], Xv[:, 31, :],
                    start=True, stop=False,
                )
                nc.tensor.matmul(
                    C2[:, 512:1024], WD[:, 1 * 128 : 2 * 128], Xv[:, 0, :],
                    start=False, stop=False,
                )
                nc.tensor.matmul(
                    C2[:, 512:1024], WD[:, 2 * 128 : 3 * 128], Xv[:, 1, :],
                    start=False, stop=True,
                )
                nc.scalar.activation(
                    Y[:, 0 : 2 * D], C2[:, :], AF.Copy, bias=bias + 1.0
                )
                nc.vector.memset(Y[0:1, 0:D], float("-inf"))

            # maxpool (pre-activation; ELU is monotonic), all shifted +1:
            # P[8p + j] = max(y3[2j], y3[2j+1], y3[2j+2]) = v + 1; then
            # out+1 = max(exp(min(v,0)), v+1), via one 4x dual-op
            # tensor_scalar (m = min(P-1, 0)), one ScalarE Exp, one 2x
            # tensor_tensor max. Stores go out fp16 on the sync (HWDGE)
            # queue, parallel to the SWDGE input queue.
            def emit_pool(tile, ja, jb):
                b, X, Y, P, R = tile
                y3 = Y[:, :].rearrange("p (q d) -> p q d", d=D)
                p3 = P[:, :].rearrange("p (j d) -> p j d", d=D)
                ps = p3[:, ja:jb, :]
                pf = P[:, ja * D : jb * D]
                rs = R[:, ja * D : jb * D]
                nc.vector.tensor_tensor(
                    ps,
                    y3[:, 2 * ja : 2 * jb - 1 : 2, :],
                    y3[:, 2 * ja + 1 : 2 * jb : 2, :],
                    op=ALU.max,
                )
                nc.vector.tensor_tensor(
                    ps, ps, y3[:, 2 * ja + 2 : 2 * jb + 1 : 2, :], op=ALU.max
                )
                nc.vector.tensor_scalar(
                    rs, pf, -1.0, 0.0, op0=ALU.add, op1=ALU.min
                )
                nc.scalar.activation(rs, rs, AF.Exp)
                nc.vector.tensor_tensor(rs, rs, pf, op=ALU.max)
                nc.sync.dma_start(
                    out=bass.AP(
                        y,
                        b * ybat + ja * xrow,
                        [[JT * xrow, 128], [1, (jb - ja) * xrow]],
                    ),
                    in_=rs,
                )

            # Emission follows chunk-readiness order: the sequenced engines
            # (ScalarE evicts in particular) execute in program order, so an
            # early-ready op emitted after a late-gated one head-of-line
            # blocks the engine.
            stages = [
                ("w", 16, 17),   # ready after chunk (29,32)
                ("w", 2, 6),     # ready after chunk (0,11)
                ("bd", 0, 0),    # ready after chunks (29,32)+(0,11)
                ("p", 0, 1),
                ("w", 6, 10),    # after chunk (11,19)
                ("p", 1, 4),
                ("w", 10, 13),   # after chunk (19,25)
                ("p", 4, 6),
                ("wv", 13, 14),  # after chunk (25,27)
                ("wv", 14, 15),  # after chunk (27,28)
                ("p", 6, 7),
                ("wv", 15, 16),  # after the final chunk (28,29)
                ("p", 7, 8),
            ]
            for kind, a_, b_ in stages:
                for tile in tiles:
                    if kind == "w":
                        emit_wave(tile, a_, b_)
                    elif kind == "wv":
                        emit_wave(tile, a_, b_, evict_on_v=True)
                    elif kind == "bd":
                        emit_boundary(tile)
                    else:
                        emit_pool(tile, a_, b_)
    return nc


def kernel(x: np.ndarray, w: np.ndarray, b: np.ndarray) -> np.ndarray:
    global LAST_RESULT
    w = np.asarray(w, dtype=np.float32)
    bb = np.asarray(b, dtype=np.float32)
    key = (float(w[0]), float(w[1]), float(w[2]), float(bb[0]))
    if key not in _cache:
        _cache[key] = _build(*key)
    nc = _cache[key]

    x = np.asarray(x, dtype=np.float32)
    assert x.shape == (B, L, D), x.shape
    wdiag = np.concatenate(
        [np.eye(128, dtype=np.float16) * np.float16(w[k]) for k in range(3)]
        + [
            np.eye(128, k=1, dtype=np.float16) * np.float16(w[k])
            for k in range(3)
        ],
        axis=1,
    )
    in_maps = [
        {
            "x": np.ascontiguousarray(x[c * BPC : (c + 1) * BPC]),
            "wd": wdiag,
        }
        for c in range(N_CORES)
    ]
    res = run_bass_kernel_spmd(nc, in_maps, core_ids=list(range(N_CORES)))
    LAST_RESULT = res
    out = np.concatenate([r["y"] for r in res.results], axis=0)
    # device computes out+1 in fp16 (see _build); undo the shift here
    return out.astype(np.float32) - 1.0


# revision 36
# speedup vs baseline: 1.2035x; 1.2024x over previous
"""Trainium2 Bass kernel for nn_DistillingLayer: per-channel shared-weight
Conv1d(k=3, stride=2, pad=1) + ELU + MaxPool1d(k=3, stride=2, pad=1) over
x:(16, 4096, 512) f32 -> out:(16, 1024, 512) f32.

Strategy (TensorE conv + fp16 streaming, DMA-roofline focused)
--------------------------------------------------------------
- Data-parallel over batch: 8 cores x 2 batches each. No communication.
- The kernel is HBM-bound: it must read 16.8 MB of f32 input per core
  (~330-420 GB/s/NC observed). Everything else is arranged so the SWDGE
  input stream never waits: measured e2e ~= preamble (9 us) + stream
  (42-52 us) + drain tail (~12 us) + exit barrier (~9 us).
- Layout: L in the SBUF free dimension; one tile per batch; partition p
  owns exactly rows [32p, 32p+32) x D=512 channels (no halo reload).
- Input DMAs run on the gpsimd (SWDGE) queue and cast f32->fp16 in the
  SDMA datapath. The two tiles' row-chunks are interleaved and ordered
  so compute dependencies unlock smoothly; the final chunk is a single
  row feeding one short conv+pool+store chain.
- Conv runs on the otherwise-idle TensorE: diag(w_k) stationaries make
  matmuls partition-preserving elementwise scales, accumulated in PSUM
  (fp32); w_k*eye(k=1) shift stationaries route partition p-1's last
  rows into p's two boundary conv rows, replacing a 3-row halo reload
  (drops input traffic 8.6%). ScalarE evicts PSUM -> fp16 SBUF with
  the conv bias (+1, see below) folded into the activation bias.
- ELU is monotonic, so maxpool commutes: pool the pre-activation conv
  rows (two VectorE 2x tensor_tensor max passes), then ELU once on the
  pooled rows. The whole pipeline computes out+1 (conv bias shifted by
  +1; host subtracts 1): out+1 = max(exp(min(v,0)), v+1) needs only a
  4x dual-op tensor_scalar (min(P-1,0)), one ScalarE Exp, and a 2x
  tensor_tensor max. scalar_tensor_tensor is avoided everywhere (it
  only has a 1x DVE uop).
- Outputs are stored as fp16 via the sync (HWDGE) queue - it runs in
  parallel with the SWDGE input queue, so stores never block loads -
  and upcast to f32 on the host (absmax-scaled error ~8e-4, gate 2e-2).
- Weights/bias are baked as immediates; the compiled module is cached
  per (w, b) value.

Toolchain workaround (see inline comment): a BIR post-pass splits
multi-wait instructions — this walrus build allows one sync wait per
instruction.
"""

import json as _json
import os
import sys

import numpy as np

for _p in ("/opt/trn_rl_repo", "/root/.axon_site/_ro/trn_rl_repo"):
    if os.path.isdir(_p) and _p not in sys.path:
        sys.path.append(_p)

import concourse.bass as bass
import concourse.bass2jax as bass2jax
import concourse.bass_utils as bass_utils
import concourse.mybir as mybir
from concourse.bass_utils import run_bass_kernel_spmd
from concourse.tile import TileContext

# ---------------------------------------------------------------------------
# REQUIRED workaround: this container's walrus build rejects instructions
# carrying more than one sync wait ("Too many sync wait commands" in
# setupSyncWait). Tile's scheduler freely attaches several waits to one
# instruction, so post-process the BIR JSON before compile: hoist all but the
# last wait onto same-engine NoOps inserted just before the instruction
# (per-engine program order makes sequential waits equivalent to a
# multi-wait).
# ---------------------------------------------------------------------------

_orig_compile_bir_kernel = bass_utils.compile_bir_kernel


def _split_multi_waits(bir_json: bytes) -> bytes:
    j = _json.loads(bir_json)
    ctr = 0
    changed = False
    for fn in j["functions"]:
        for bb in fn["blocks"]:
            out = []
            for ins in bb["instructions"]:
                si = ins.get("sync_info")
                waits = (si.get("on_wait") or []) if si else []
                if len(waits) > 1:
                    changed = True
                    for w in waits[:-1]:
                        ctr += 1
                        out.append(
                            {
                                "debug": ins.get("debug", 0),
                                "engine": ins["engine"],
                                "ins": [],
                                "outs": [],
                                "name": f"waitsplit-{ctr}",
                                "opcode": "NoOp",
                                "text_hint": "waitsplit",
                                "sync_info": {"on_update": [], "on_wait": [w]},
                            }
                        )
                    si["on_wait"] = [waits[-1]]
                out.append(ins)
            bb["instructions"] = out
    if not changed:
        return bir_json
    return _json.dumps(j).encode()


def _patched_compile_bir_kernel(bir_json, tmpdir, neff_name="file.neff"):
    return _orig_compile_bir_kernel(_split_multi_waits(bir_json), tmpdir, neff_name)


bass_utils.compile_bir_kernel = _patched_compile_bir_kernel
bass2jax.compile_bir_kernel = _patched_compile_bir_kernel

# The first TileContext exit barrier's per-engine drains are redundant (the
# tail waits already cover all completions); use the cheap sequencer-level
# variant there. The SECOND barrier stays full — its drains restore
# engine/queue state so the loaded NEFF can re-execute.
try:
    from concourse.vector_clock import ScopedClock as _ScopedClock

    def _tail_drain_and_barrier(self, tick_clock, wait_clock):
        drain_inst = self.nc.sync.drain()
        wait_clock.add_sem_waits(
            drain_inst.ins, _ScopedClock({None: tick_clock.global_clock})
        )
        self.nc.all_engine_barrier(sem_only=True)
        assert self.sems is not None
        popped = self.nc._tile_sem_poison_stack.pop()
        assert popped is self._sem_poison
        # Skip the device-side dma_reset/sem_clear of
        # clear_and_free_semaphores: the bass preamble re-clears the full
        # semaphore range at the start of every execution, so exit-time
        # clears are redundant (re-execution correctness verified by
        # running the kernel twice in one process). Keep the host-side
        # allocator bookkeeping.
        sem_nums = [s.num for s in self.sems.allocated().values()]
        self.nc._state.prepend_free_semaphores(sem_nums)
        for poison_set in self.nc._tile_sem_poison_stack:
            poison_set.update(sem_nums)
        self.nc.all_engine_barrier(sem_only=True)

    TileContext._drain_and_barrier = _tail_drain_and_barrier
except Exception:
    pass

# ---------------------------------------------------------------------------

N_CORES = 8
B, L, D = 16, 4096, 512
BPC = B // N_CORES  # batches per core
LC = L // 2         # conv output length
LP = LC // 2        # pool output length
S = 32              # input L-rows owned per partition (128 * 32 = 4096)
Q = S // 2 + 1      # conv rows per partition (incl. 1 left-halo conv row)
JT = S // 4         # pool-output rows per partition

F32 = mybir.dt.float32
F16 = mybir.dt.float16
ALU = mybir.AluOpType
AF = mybir.ActivationFunctionType

_cache: dict = {}

# Exposed for test harnesses: the BassKernelResults of the last run.
LAST_RESULT = None


def _build(w0: float, w1: float, w2: float, bias: float) -> bass.Bass:
    nc = bass.Bass()
    # x is the raw unpadded input: partition p of a tile owns exactly rows
    # [32p, 32p+32) -- no halo rows are loaded (see the shift matrices).
    x = nc.dram_tensor("x", [BPC, L, D], F32, kind="ExternalInput")
    # wd holds six 128x128 stationary matrices (fp16): w_k * I for k=0..2
    # (diag(w) @ X == w * X elementwise, partition-preserving) and
    # w_k * eye(k=1), whose matmul routes partition p-1's row to partition
    # p -- used for the two boundary conv rows whose taps live in the
    # previous partition, replacing a 3-row halo reload from HBM.
    wd = nc.dram_tensor("wd", [128, 6 * 128], F16, kind="ExternalInput")
    y = nc.dram_tensor("y", [BPC, LP, D], F16, kind="ExternalOutput")

    xrow = D              # elements per L-row
    xbat = L * D          # elements per input batch
    ybat = LP * D

    with TileContext(nc) as tc:
        with (
            tc.tile_pool(name="xp", bufs=2) as xp,
            tc.tile_pool(name="yp", bufs=2) as yp,
            tc.tile_pool(name="wp", bufs=1) as wp,
            tc.tile_pool(name="cp", bufs=2, space="PSUM") as cp,
            tc.tile_pool(name="pp", bufs=2) as pp,
            tc.tile_pool(name="rp", bufs=2) as rp,
        ):
            # The six stationary matrices, loaded once up front on the sync
            # (HWDGE) queue so the SWDGE input stream is not delayed.
            WD = wp.tile([128, 6 * 128], F16)
            nc.sync.dma_start(
                out=WD[:, :],
                in_=bass.AP(wd, 0, [[6 * 128, 128], [1, 6 * 128]]),
            )
            # Input row-chunks, conv q-waves and pool j-segments are aligned
            # so each conv wave only needs already-landed chunks (conv row q
            # taps local rows [2qa, 2qb+1)) and each pool segment only needs
            # finished conv rows (q in [2ja, 2jb+1)). The two batch tiles'
            # chunks are INTERLEAVED in the SWDGE stream, so each tile's
            # compute spreads across the whole stream and only one short
            # wave+pool chain hangs off the final chunk (tile 1's pool seg
            # (7,8) via conv q=16).
            # Conv wave (qa,qb) with qa>=2 taps local rows [2qa-3, 2qb-2);
            # the boundary wave (0,2) taps the previous partition's rows
            # 29-31 (routed cross-partition by the shift stationaries) plus
            # local rows 0-1. Rows 29-31 are loaded FIRST, so both the
            # boundary wave and q=16 unlock early; the final chunk is the
            # single row 28, which only conv row q=15 needs -- the
            # post-stream tail is one 3-matmul + FD-512 evict/pool/store
            # chain per tile.
            chunks = [(29, 32), (0, 11), (11, 19), (19, 25), (25, 27), (27, 28), (28, 29)]

            tiles = []
            for b in range(BPC):
                # Partition p holds exactly its own rows [32p, 32p+32).
                X = xp.tile([128, S * D], F16)
                Y = yp.tile([128, Q * D], F16)
                P = pp.tile([128, JT * D], F16)
                R = rp.tile([128, JT * D], F16)
                tiles.append((b, X, Y, P, R))

            for ci in range(len(chunks)):
                r0, r1 = chunks[ci]
                for b, X, Y, P, R in tiles:
                    nc.gpsimd.dma_start(
                        out=X[:, r0 * D : r1 * D],
                        in_=bass.AP(
                            x,
                            b * xbat + r0 * xrow,
                            [[S * xrow, 128], [1, (r1 - r0) * xrow]],
                        ),
                    )

            # conv, shifted by +1 (the host subtracts 1 from the final
            # output): partition p's conv row q (local) is
            # c[16p - 1 + q] = w0*x[2q-3] + w1*x[2q-2] + w2*x[2q-1] + bias+1
            # (x indices local to the partition's 32-row strip).
            # The taps run on the otherwise-idle TensorE: diag(w_k) as the
            # stationary makes a matmul a partition-preserving elementwise
            # scale, and the three taps accumulate in a PSUM bank (fp32).
            # Matmuls are grouped by tap so the stationary is swapped 3x per
            # wave, not per row. ScalarE then evicts PSUM -> fp16 SBUF,
            # folding in bias+1 via the activation bias.
            def emit_wave(tile, qa, qb, evict_on_v=False):
                b, X, Y, P, R = tile
                nq = qb - qa
                Xv = X[:, :].rearrange("p (r d) -> p r d", d=D)
                C4 = cp.tile([128, nq * 512], F32, tag="cw")
                for k in range(3):
                    Wk = WD[:, k * 128 : (k + 1) * 128]
                    for q in range(qa, qb):
                        nc.tensor.matmul(
                            C4[:, (q - qa) * 512 : (q - qa + 1) * 512],
                            Wk,
                            Xv[:, 2 * q - 3 + k, :],
                            start=(k == 0),
                            stop=(k == 2),
                        )
                if evict_on_v:
                    # Late waves evict on DVE (bias via tensor_scalar add)
                    # so the stream-end eviction+exp backlog splits across
                    # ScalarE and VectorE instead of serializing on ScalarE.
                    nc.vector.tensor_scalar(
                        Y[:, qa * D : qb * D], C4[:, :], bias + 1.0, None,
                        op0=ALU.add,
                    )
                else:
                    nc.scalar.activation(
                        Y[:, qa * D : qb * D], C4[:, :], AF.Copy, bias=bias + 1.0
                    )

            # Boundary wave (0,2): conv rows q=0,1 of partition p tap rows
            # 29-31 of partition p-1 (routed by the shift stationaries
            # WD[:, (3+k)*128:]) plus local rows 0,1. Partition 0's q=0 gets
            # all-zero shift input (= the pool's excluded left pad), so it
            # is overwritten with -inf after eviction; its q=1 correctly
            # sees zero for the conv's left pad x[-1].
            def emit_boundary(tile):
                b, X, Y, P, R = tile
                Xv = X[:, :].rearrange("p (r d) -> p r d", d=D)
                C2 = cp.tile([128, 2 * 512], F32, tag="cw")
                for k in range(3):
                    Sk = WD[:, (3 + k) * 128 : (4 + k) * 128]
                    nc.tensor.matmul(
                        C2[:, 0:512],
                        Sk,
                        Xv[:, 29 + k, :],
                        start=(k == 0),
                        stop=(k == 2),
                    )
                nc.tensor.matmul(
                    C2[:, 512:1024], WD[:, 3 * 128 : 4 * 128], Xv[:, 31, :],
                    start=True, stop=False,
                )
                nc.tensor.matmul(
                    C2[:, 512:1024], WD[:, 1 * 128 : 2 * 128], Xv[:, 0, :],
                    start=False, stop=False,
                )
                nc.tensor.matmul(
                    C2[:, 512:1024], WD[:, 2 * 128 : 3 * 128], Xv[:, 1, :],
                    start=False, stop=True,
                )
                nc.scalar.activation(
                    Y[:, 0 : 2 * D], C2[:, :], AF.Copy, bias=bias + 1.0
                )
                nc.vector.memset(Y[0:1, 0:D], float("-inf"))

            # maxpool (pre-activation; ELU is monotonic), all shifted +1:
            # P[8p + j] = max(y3[2j], y3[2j+1], y3[2j+2]) = v + 1; then
            # out+1 = max(exp(min(v,0)), v+1), via one 4x dual-op
            # tensor_scalar (m = min(P-1, 0)), one ScalarE Exp, one 2x
            # tensor_tensor max. Stores go out fp16 on the sync (HWDGE)
            # queue, parallel to the SWDGE input queue.
            def emit_pool(tile, ja, jb):
                b, X, Y, P, R = tile
                y3 = Y[:, :].rearrange("p (q d) -> p q d", d=D)
                p3 = P[:, :].rearrange("p (j d) -> p j d", d=D)
                ps = p3[:, ja:jb, :]
                pf = P[:, ja * D : jb * D]
                rs = R[:, ja * D : jb * D]
                nc.vector.tensor_tensor(
                    ps,
                    y3[:, 2 * ja : 2 * jb - 1 : 2, :],
                    y3[:, 2 * ja + 1 : 2 * jb : 2, :],
                    op=ALU.max,
                )
                nc.vector.tensor_tensor(
                    ps, ps, y3[:, 2 * ja + 2 : 2 * jb + 1 : 2, :], op=ALU.max
                )
                nc.vector.tensor_scalar(
                    rs, pf, -1.0, 0.0, op0=ALU.add, op1=ALU.min
                )
                nc.scalar.activation(rs, rs, AF.Exp)
                nc.vector.tensor_tensor(rs, rs, pf, op=ALU.max)
                nc.sync.dma_start(
                    out=bass.AP(
                        y,
                        b * ybat + ja * xrow,
                        [[JT * xrow, 128], [1, (jb - ja) * xrow]],
                    ),
                    in_=rs,
                )

            # Emission follows chunk-readiness order: the sequenced engines
            # (ScalarE evicts in particular) execute in program order, so an
            # early-ready op emitted after a late-gated one head-of-line
            # blocks the engine.
            stages = [
                ("w", 16, 17),   # ready after chunk (29,32)
                ("w", 2, 6),     # ready after chunk (0,11)
                ("bd", 0, 0),    # ready after chunks (29,32)+(0,11)
                ("p", 0, 1),
                ("w", 6, 10),    # after chunk (11,19)
                ("p", 1, 4),
                ("w", 10, 13),   # after chunk (19,25)
                ("p", 4, 6),
                ("w", 13, 14),   # after chunk (25,27)
                ("w", 14, 15),   # after chunk (27,28)
                ("p", 6, 7),
                ("w", 15, 16),   # after the final chunk (28,29)
                ("p", 7, 8),
            ]
            for kind, a_, b_ in stages:
                for tile in tiles:
                    if kind == "w":
                        emit_wave(tile, a_, b_)
                    elif kind == "wv":
                        emit_wave(tile, a_, b_, evict_on_v=True)
                    elif kind == "bd":
                        emit_boundary(tile)
                    else:
                        emit_pool(tile, a_, b_)
    return nc


def kernel(x: np.ndarray, w: np.ndarray, b: np.ndarray) -> np.ndarray:
    global LAST_RESULT
    w = np.asarray(w, dtype=np.float32)
    bb = np.asarray(b, dtype=np.float32)
    key = (float(w[0]), float(w[1]), float(w[2]), float(bb[0]))
    if key not in _cache:
        _cache[key] = _build(*key)
    nc = _cache[key]

    x = np.asarray(x, dtype=np.float32)
    assert x.shape == (B, L, D), x.shape
    wdiag = np.concatenate(
        [np.eye(128, dtype=np.float16) * np.float16(w[k]) for k in range(3)]
        + [
            np.eye(128, k=1, dtype=np.float16) * np.float16(w[k])
            for k in range(3)
        ],
        axis=1,
    )
    in_maps = [
        {
            "x": np.ascontiguousarray(x[c * BPC : (c + 1) * BPC]),
            "wd": wdiag,
        }
        for c in range(N_CORES)
    ]
    res = run_bass_kernel_spmd(nc, in_maps, core_ids=list(range(N_CORES)))
    LAST_RESULT = res
    out = np.concatenate([r["y"] for r in res.results], axis=0)
    # device computes out+1 in fp16 (see _build); undo the shift here
    return out.astype(np.float32) - 1.0
